# revision 10
# baseline (speedup 1.0000x reference)
"""DeformableConv1D Trainium2 kernel.

Math: the reference reduces to
    offset = conv1d(x, Wconv) + bconv
    m = mean(offset);  scale_k = relu(1 - |m + R_k|);  s = sum_k Wdef[k]*scale_k
    out = conv1d(s*x, Wconv) + bconv = s * conv_nobias(x) + bconv

mean(offset) only needs per-channel sums of x (the tiny edge corrections use
8 rows per batch and are folded into a host-precomputed constant).

Device program (per core, data-parallel over batch: 2 batches/core):

  Phase 1: x viewed as X[q, (s',c)] (q = 4-step block; 128 = 4 sub-steps x 32
  ch). Load tiles [128, 2048] fp32 put 64 consecutive timesteps (16 q) on each
  partition: one contiguous 8 KiB DMA line per partition. Each PE transpose of
  a [128,128] slice n yields psum [(s',c), cols p] holding q = blk*2048+16p+n,
  so the resident fp16 xt is *scrambled*: storage col blk*2048 + n*128 + p
  <-> q = blk*2048 + 16p + n. Drains are contiguous on both sides and carry
  the per-(s',c) channel sums via accum_out.

  Phase 2 (overlapped with phase 1): polyphase conv with the WEIGHTS
  stationary. Per 512-q psum bank the A matmul streams the strided xt view
  (n:16, p:32) and the B matmuls stream the same view shifted by one q
  (storage +128, plus a 32-col fixup from the n=0 row, plus a 1-col
  cross-block fixup). Psum columns keep the scrambled (n, p) order; drains
  cast to fp16 stages UNSCALED (not gated on the AllReduce), so matmuls and
  drains run during phase 1 and the collective latency.

  AllReduce 4 B of channel-sum dot; s computed on device; stages scaled in
  place by s and stored (transposed fp16 layout, per-partition contiguous).
  The host unscrambles/de-transposes and upcasts to fp32.

Sharding: data-parallel over batch (2 batches per core x 8 cores).
bconv is all-zero in this problem; if not, it is added on the host.
"""

import numpy as np

import concourse.bacc as bacc
import concourse.bass as bass
import concourse.mybir as mybir
import concourse.tile as tile
from concourse.bass_utils import run_bass_kernel_spmd

FP = mybir.dt.float32
CONV_DT = mybir.dt.float16

N_CORES = 8
B_TOTAL = 16
T = 65536
C = 32
F = 32
K = 5

BPC = B_TOTAL // N_CORES      # batches per core
Q = T // 4                    # q blocks per batch (16384)
QT = Q * BPC                  # xt columns per core (32768)
BLKQ = 2048                   # q per load tile / xt block
NBLK = QT // BLKQ             # load tiles per core (16)
HQ = 1024                     # q per psum tile / drain
STQ = 4096                    # q per staged store
NST = QT // STQ               # stages per core (8)


def build_kernel():
    nc = bacc.Bacc(
        "TRN2",
        target_bir_lowering=False,
        debug=False,
        enable_asserts=False,
        num_devices=N_CORES,
    )
    x = nc.dram_tensor("x", [BPC, T, C], FP, kind="ExternalInput").ap()
    wa = nc.dram_tensor("wa", [128, 128], CONV_DT, kind="ExternalInput").ap()
    wb = nc.dram_tensor("wb", [128, 128], CONV_DT, kind="ExternalInput").ap()
    ident = nc.dram_tensor("ident", [128, 128], CONV_DT, kind="ExternalInput").ap()
    qcv = nc.dram_tensor("qcv", [128, 1], FP, kind="ExternalInput").ap()
    c1 = nc.dram_tensor("c1", [1, 1], FP, kind="ExternalInput").ap()
    taps = nc.dram_tensor("taps", [1, K], FP, kind="ExternalInput").ap()
    wdef = nc.dram_tensor("wdef", [1, K], FP, kind="ExternalInput").ap()
    out = nc.dram_tensor("out", [BPC, 128, Q], CONV_DT, kind="ExternalOutput").ap()

    # load tiles: partition p holds 64 consecutive timesteps (8 KiB line)
    x_v = x.rearrange("g (tb p r) c -> g tb p (r c)", p=128, r=64)

    with tile.TileContext(nc) as tc:
        with (
            tc.tile_pool(name="res", bufs=1) as res_pool,
            tc.tile_pool(name="xload", bufs=3) as xload_pool,
            tc.tile_pool(name="stage", bufs=1) as stage_pool,
            tc.tile_pool(name="consts", bufs=1) as cpool,
            tc.tile_pool(name="ps1", bufs=2, space="PSUM") as ps1_pool,
            tc.tile_pool(name="ps2", bufs=3, space="PSUM") as ps2_pool,
            tc.tile_pool(name="dram", bufs=1, space="DRAM") as dram_pool,
        ):
            # resident transposed x, scrambled:
            # xt[:, blk*2048 + n*128 + p] = X[blk*2048 + 16p + n, (s'c)]
            xt = res_pool.tile([128, QT], CONV_DT)
            xtv = xt.rearrange("k (blk n p) -> k blk n p", n=16, p=128)

            identity = cpool.tile([128, 128], CONV_DT)
            nc.gpsimd.dma_start(identity[:], ident[:])
            wa_t = cpool.tile([128, 128], CONV_DT)
            nc.gpsimd.dma_start(wa_t[:], wa[:])
            wb_t = cpool.tile([128, 128], CONV_DT)
            nc.gpsimd.dma_start(wb_t[:], wb[:])
            qcv_t = cpool.tile([128, 1], FP)
            nc.gpsimd.dma_start(qcv_t[:], qcv[:])
            c1_t = cpool.tile([1, 1], FP)
            nc.gpsimd.dma_start(c1_t[:], c1[:])
            taps_t = cpool.tile([1, K], FP)
            nc.gpsimd.dma_start(taps_t[:], taps[:])
            wdef_t = cpool.tile([1, K], FP)
            nc.gpsimd.dma_start(wdef_t[:], wdef[:])

            nacc = NBLK * (BLKQ // HQ)
            acc = cpool.tile([128, nacc], FP)
            nc.vector.memset(acc[:], 0.0)

            stages = [
                stage_pool.tile([128, STQ], CONV_DT, name=f"stg{k}")
                for k in range(NST)
            ]

            idr = 0
            nc2 = 0

            def emit_conv_group(blk, hh):
                """Conv for q in [blk*2048 + hh*1024, +1024): 2 psum banks."""
                nonlocal nc2
                po = ps2_pool.tile([128, HQ], FP, name="po")
                for hl in range(2):
                    h = 2 * hh + hl
                    p0 = 32 * h
                    nc.tensor.matmul(
                        po[:, hl * 512 : hl * 512 + 512],
                        wa_t[:], xtv[:, blk, :, p0 : p0 + 32],
                        start=True, stop=False,
                    )
                for hl in range(2):
                    h = 2 * hh + hl
                    p0 = 32 * h
                    nc.tensor.matmul(
                        po[:, hl * 512 : hl * 512 + 480],
                        wb_t[:], xtv[:, blk, 1:16, p0 : p0 + 32],
                        start=False, stop=False,
                    )
                    if p0 + 33 <= 128:
                        nc.tensor.matmul(
                            po[:, hl * 512 + 480 : hl * 512 + 512],
                            wb_t[:], xt[:, blk * BLKQ + p0 + 1 : blk * BLKQ + p0 + 33],
                            start=False, stop=True,
                        )
                    else:  # h == 3: 31-col fixup + 1-col cross-block fixup
                        nc.tensor.matmul(
                            po[:, hl * 512 + 480 : hl * 512 + 511],
                            wb_t[:], xt[:, blk * BLKQ + p0 + 1 : blk * BLKQ + p0 + 32],
                            start=False, stop=False,
                        )
                        if (blk + 1) % (NBLK // BPC) != 0:
                            nc.tensor.matmul(
                                po[:, hl * 512 + 511 : hl * 512 + 512],
                                wb_t[:], xt[:, (blk + 1) * BLKQ : (blk + 1) * BLKQ + 1],
                                start=False, stop=True,
                            )
                # drain unscaled to the resident stage (not gated on AllReduce)
                gi = blk * 2 + hh
                stg = stages[gi // 4]
                dst = stg[:, (gi % 4) * HQ : (gi % 4 + 1) * HQ]
                if nc2 % 2 == 0:
                    nc.vector.tensor_copy(dst, po[:])
                else:
                    nc.scalar.activation(
                        dst, po[:], mybir.ActivationFunctionType.Copy
                    )
                nc2 += 1

            # ---- Phase 1 (+ overlapped conv): load, transpose, drain ----
            for blk in range(NBLK):
                g, tb = divmod(blk, NBLK // BPC)
                lt = xload_pool.tile([128, BLKQ], CONV_DT)
                nc.gpsimd.dma_start(lt[:], x_v[g, tb])
                for hh in range(BLKQ // HQ):
                    pt = ps1_pool.tile([128, HQ], CONV_DT, name="pt")
                    for nl in range(HQ // 128):
                        n = hh * (HQ // 128) + nl
                        nc.tensor.transpose(
                            pt[:, nl * 128 : (nl + 1) * 128],
                            lt[:, n * 128 : (n + 1) * 128],
                            identity[:],
                        )
                    col0 = blk * BLKQ + hh * HQ
                    dst = xt[:, col0 : col0 + HQ]
                    if idr % 2 == 0:
                        nc.scalar.activation(
                            dst, pt[:], mybir.ActivationFunctionType.Copy,
                            accum_out=acc[:, idr : idr + 1],
                        )
                    else:
                        nc.vector.tensor_scalar(
                            dst, pt[:], 1.0, 0.0,
                            op0=mybir.AluOpType.mult,
                            op1=mybir.AluOpType.add,
                            accum_out=acc[:, idr : idr + 1],
                        )
                    idr += 1
                if blk >= 1:
                    emit_conv_group(blk - 1, 0)
                    emit_conv_group(blk - 1, 1)

            # per-(s',c) partial sums -> local dot with qcv
            localsum = cpool.tile([128, 1], FP)
            nc.vector.tensor_reduce(
                localsum[:], acc[:], axis=mybir.AxisListType.X,
                op=mybir.AluOpType.add,
            )
            psd = ps2_pool.tile([128, HQ], FP, name="po")
            nc.tensor.matmul(psd[0:1, 0:1], localsum[:], qcv_t[:])
            mloc = cpool.tile([1, 1], FP)
            nc.vector.tensor_copy(mloc[:], psd[0:1, 0:1])

            # ---- AllReduce the scalar across cores ----
            ar_in = dram_pool.tile([1, 1], FP)
            ar_out = dram_pool.tile([1, 1], FP, addr_space="Shared")
            nc.gpsimd.dma_start(ar_in[:], mloc[:])
            nc.gpsimd.collective_compute(
                "AllReduce",
                mybir.AluOpType.add,
                replica_groups=[list(range(N_CORES))],
                ins=[ar_in.opt()],
                outs=[ar_out.opt()],
            )
            mg = cpool.tile([1, 1], FP)
            nc.sync.dma_start(mg[:], ar_out[:])

            # last block's conv (overlaps the collective)
            emit_conv_group(NBLK - 1, 0)
            emit_conv_group(NBLK - 1, 1)

            # ---- s = sum_k Wdef[k] * relu(1 - |m + c1 + R_k|), broadcast ----
            m1 = cpool.tile([1, 1], FP)
            nc.vector.tensor_tensor(m1[:], mg[:], c1_t[:], op=mybir.AluOpType.add)
            t1 = cpool.tile([1, K], FP)
            nc.vector.tensor_scalar_add(t1[:], taps_t[:], m1[:])
            t2 = cpool.tile([1, K], FP)
            nc.scalar.activation(t2[:], t1[:], mybir.ActivationFunctionType.Abs)
            t3 = cpool.tile([1, K], FP)
            nc.vector.tensor_scalar(
                t3[:], t2[:], -1.0, 1.0,
                op0=mybir.AluOpType.mult, op1=mybir.AluOpType.add,
            )
            t4 = cpool.tile([1, K], FP)
            nc.vector.tensor_scalar_max(t4[:], t3[:], 0.0)
            t5 = cpool.tile([1, K], FP)
            nc.vector.tensor_tensor(t5[:], t4[:], wdef_t[:], op=mybir.AluOpType.mult)
            s11 = cpool.tile([1, 1], FP)
            nc.vector.tensor_reduce(
                s11[:], t5[:], axis=mybir.AxisListType.X, op=mybir.AluOpType.add
            )
            s_b = cpool.tile([128, 1], FP)
            nc.gpsimd.partition_broadcast(s_b[:], s11[:])

            # ---- scale stages in place by s, then store ----
            # half-stage granularity; DVE-heavy engine split (ACT is ~2.6x
            # slower on fp16) so stores start early and pipeline
            for k in range(NST):
                stg = stages[k]
                g, kb = divmod(k, NST // BPC)
                for half in range(2):
                    sl = stg[:, half * (STQ // 2) : (half + 1) * (STQ // 2)]
                    if (2 * k + half) % 3 == 2:
                        nc.scalar.activation(
                            sl, sl, mybir.ActivationFunctionType.Copy,
                            scale=s_b[:],
                        )
                    else:
                        nc.vector.tensor_scalar_mul(sl, sl, s_b[:])
                    qs = kb * STQ + half * (STQ // 2)
                    nc.sync.dma_start(
                        out[g, :, qs : qs + STQ // 2],
                        sl,
                    )

    nc.compile()
    return nc


_NC_CACHE = None
_LAST_IN_MAPS = None


def _get_nc():
    global _NC_CACHE
    if _NC_CACHE is None:
        _NC_CACHE = build_kernel()
    return _NC_CACHE


def _host_consts(x, Wconv, bconv):
    Tout = T - K + 1
    Ntot = B_TOTAL * Tout * F
    Wsum = Wconv.sum(axis=2).astype(np.float64)  # (K, C)
    head = x[:, : K - 1, :].astype(np.float64).sum(axis=0)  # (4, C)
    tail = x[:, T - (K - 1) :, :].astype(np.float64).sum(axis=0)  # (4, C)
    pre = np.concatenate([np.zeros((1, C)), np.cumsum(head, axis=0)], axis=0)
    suf = np.concatenate([np.zeros((1, C)), np.cumsum(tail[::-1], axis=0)], axis=0)
    edge = 0.0
    for k in range(K):
        edge += (Wsum[k] * (pre[k] + suf[K - 1 - k])).sum()
    qc = (Wsum.sum(axis=0) / Ntot).astype(np.float32)
    qcvec = np.tile(qc, 4).reshape(128, 1)
    c1 = np.float32(-edge / Ntot + float(np.mean(bconv)))
    return qcvec, np.array([[c1]], np.float32)


def _build_ab(Wconv):
    A = np.zeros((128, 128), np.float32)
    B = np.zeros((128, 128), np.float32)
    for sp in range(4):
        for so in range(4):
            k = sp - so
            if 0 <= k < K:
                A[sp * 32 : (sp + 1) * 32, so * 32 : (so + 1) * 32] = Wconv[k]
            k2 = sp - so + 4
            if 0 <= k2 < K:
                B[sp * 32 : (sp + 1) * 32, so * 32 : (so + 1) * 32] = Wconv[k2]
    return A.astype(np.float16), B.astype(np.float16)


def kernel(x, Wconv, bconv, Wdef):
    x = np.ascontiguousarray(np.asarray(x, np.float32))
    Wconv = np.asarray(Wconv, np.float32)
    bconv = np.asarray(bconv, np.float32)
    Wdef = np.asarray(Wdef, np.float32)

    nc = _get_nc()
    A, B = _build_ab(Wconv)
    qcvec, c1 = _host_consts(x, Wconv, bconv)
    ident = np.eye(128, dtype=np.float16)
    taps = (np.arange(K, dtype=np.float32) - (K // 2)).reshape(1, K)
    wdef_r = Wdef.reshape(1, K).astype(np.float32)

    in_maps = []
    for core in range(N_CORES):
        in_maps.append(
            {
                "x": x[core * BPC : (core + 1) * BPC],
                "wa": A,
                "wb": B,
                "ident": ident,
                "qcv": qcvec,
                "c1": c1,
                "taps": taps,
                "wdef": wdef_r,
            }
        )
    global _LAST_IN_MAPS
    _LAST_IN_MAPS = in_maps
    res = run_bass_kernel_spmd(nc, in_maps, list(range(N_CORES)))
    Tout = T - K + 1
    out = np.empty((B_TOTAL, Tout, F), np.float32)
    NBB = NBLK // BPC  # blocks per batch (8)
    for core in range(N_CORES):
        o = res.results[core]["out"]  # (BPC, 128, Q) fp16, scrambled cols
        # col J = blk*2048 + h*512 + n*32 + dp  <->  q = blk*2048+16*(32h+dp)+n
        o = (
            o.reshape(BPC, 4, F, NBB, 4, 16, 32)  # (g, so, f, blk, h, n, dp)
            .transpose(0, 3, 4, 6, 5, 1, 2)       # (g, blk, h, dp, n, so, f)
            .reshape(BPC, T, F)[:, :Tout, :]
        )
        out[core * BPC : (core + 1) * BPC] = o.astype(np.float32)
    if np.any(bconv):
        out += bconv.reshape(1, 1, F)
    return out
